# revision 3
# baseline (speedup 1.0000x reference)
"""Distributed AlignBlock kernel for 8 NeuronCores.

Sharding: data-parallel over B(2) x T-chunks(4 x 128) = 8 shards, one per
core. Each shard carries a causal halo (4 frames for the conv on the Q/V
side, 35 = 31 + 4 frames on the K / x_ref side). Weights are replicated.
All compute runs on the NeuronCores; the host only slices/pads inputs and
concatenates the 8 output shards.

Hardcoded problem shape: B=2, C=64, H=64, T=512, F=64, DMAX=32.
"""

import numpy as np
import jax
import jax.numpy as jnp
from functools import partial

B, C, H, T, F = 2, 64, 64, 512, 64
DMAX = 32
NCHUNK = 4          # T-chunks per batch element
TC = T // NCHUNK    # 128 frames per chunk
QHALO = 4           # conv reaches back 4 frames in t
KHALO = DMAX - 1 + QHALO  # 35: score window + conv halo
TQ = TC + QHALO     # 132 Q frames per shard
TK = TC + KHALO     # 163 K / x_ref frames per shard


def _shard_time(x, t0, halo):
    """x: (C, T, F) -> (C, TC+halo, F) covering global frames [t0-halo, t0+TC),
    zero-padded where the range dips below 0."""
    lo = t0 - halo
    if lo >= 0:
        return x[:, lo:t0 + TC, :]
    pad = -lo
    return np.concatenate(
        [np.zeros((x.shape[0], pad, x.shape[2]), x.dtype), x[:, 0:t0 + TC, :]],
        axis=1)


@partial(jax.pmap, in_axes=(0, 0, 0, 0, None, None, None, None, None, None),
         out_axes=0)
def _shard_fn(xm, xr, qmask, kmask, w_mic, b_mic, w_ref, b_ref, w_conv, b_conv):
    # xm: (C, TQ, F)  frames [t0-4, t0+128)
    # xr: (C, TK, F)  frames [t0-35, t0+128)
    Q = jnp.einsum('ctf,hc->htf', xm, w_mic) + b_mic[:, None, None]
    K = jnp.einsum('ctf,hc->htf', xr, w_ref) + b_ref[:, None, None]
    Q = Q * qmask[None, :, None]   # zero frames before global t=0 (chunk 0)
    K = K * kmask[None, :, None]
    # V[h, t', d] = <Q[h, t'], K[h, t' + d]> / sqrt(F);  t' in [0, TQ)
    # (static slices instead of gather — the neuron compiler chokes on
    # indirect loads)
    V = jnp.stack(
        [jnp.sum(Q * jax.lax.slice_in_dim(K, d, d + TQ, axis=1), axis=-1)
         for d in range(DMAX)], axis=-1) / jnp.sqrt(jnp.float32(F))
    # conv (5,3) over (t', d), H->1; valid in t' (132->128), pad d by 1
    Vp = jnp.pad(V, ((0, 0), (0, 0), (1, 1)))[None]             # (1, H, TQ, 34)
    Vc = jax.lax.conv_general_dilated(
        Vp, w_conv, window_strides=(1, 1), padding='VALID',
        dimension_numbers=('NCHW', 'OIHW', 'NCHW'))[0, 0] + b_conv[0]
    A = jax.nn.softmax(Vc, axis=-1)                             # (TC, DMAX)
    # aligned[c, t, f] = sum_d A[t, d] * xr[c, t + 4 + d, f]
    out = jnp.zeros((C, TC, F), jnp.float32)
    for d in range(DMAX):
        out = out + A[None, :, d, None] * jax.lax.slice_in_dim(
            xr, 4 + d, 4 + d + TC, axis=1)
    return out                                                  # (C, TC, F)


def kernel(x_mic, x_ref, w_mic, b_mic, w_ref, b_ref, w_conv, b_conv):
    x_mic = np.asarray(x_mic, np.float32)
    x_ref = np.asarray(x_ref, np.float32)
    xm_shards, xr_shards, qmasks, kmasks = [], [], [], []
    for b in range(B):
        for tc in range(NCHUNK):
            t0 = tc * TC
            xm_shards.append(_shard_time(x_mic[b], t0, QHALO))
            xr_shards.append(_shard_time(x_ref[b], t0, KHALO))
            qm = np.ones(TQ, np.float32)
            km = np.ones(TK, np.float32)
            if t0 - QHALO < 0:
                qm[:QHALO - t0] = 0.0
            if t0 - KHALO < 0:
                km[:KHALO - t0] = 0.0
            qmasks.append(qm)
            kmasks.append(km)
    xm = np.stack(xm_shards)          # (8, C, TQ, F)
    xr = np.stack(xr_shards)          # (8, C, TK, F)
    qm = np.stack(qmasks)             # (8, TQ)
    km = np.stack(kmasks)             # (8, TK)
    out = _shard_fn(jnp.asarray(xm), jnp.asarray(xr), jnp.asarray(qm),
                    jnp.asarray(km), jnp.asarray(w_mic), jnp.asarray(b_mic),
                    jnp.asarray(w_ref), jnp.asarray(b_ref),
                    jnp.asarray(w_conv), jnp.asarray(b_conv))
    out = np.asarray(out)             # (8, C, TC, F)
    full = np.empty((B, C, T, F), np.float32)
    for b in range(B):
        for tc in range(NCHUNK):
            full[b, :, tc * TC:(tc + 1) * TC, :] = out[b * NCHUNK + tc]
    return full


# revision 4
# speedup vs baseline: 1.3887x; 1.3887x over previous
"""Distributed AlignBlock kernel for 8 NeuronCores.

Sharding: data-parallel over B(2) x T-chunks(4 x 128) = 8 shards, one per
core. Each shard carries a causal halo (4 frames for the conv on the Q/V
side, 35 = 31 + 4 frames on the K / x_ref side). Weights are replicated.
All compute runs on the NeuronCores; the host only slices/pads inputs and
concatenates the 8 output shards. Input shards ship as bf16 (halves the
transfer), accumulation on device is f32.

Hardcoded problem shape: B=2, C=64, H=64, T=512, F=64, DMAX=32.
"""

import numpy as np
import jax
import jax.numpy as jnp
from functools import partial

B, C, H, T, F = 2, 64, 64, 512, 64
DMAX = 32
NCHUNK = 4          # T-chunks per batch element
TC = T // NCHUNK    # 128 frames per chunk
QHALO = 4           # conv reaches back 4 frames in t
KHALO = DMAX - 1 + QHALO  # 35: score window + conv halo
TQ = TC + QHALO     # 132 Q frames per shard
TK = TC + KHALO     # 163 K / x_ref frames per shard

F32 = jnp.float32


def _shard_time(x, t0, halo):
    """x: (C, T, F) -> (C, TC+halo, F) covering global frames [t0-halo, t0+TC),
    zero-padded where the range dips below 0."""
    lo = t0 - halo
    if lo >= 0:
        return x[:, lo:t0 + TC, :]
    pad = -lo
    return np.concatenate(
        [np.zeros((x.shape[0], pad, x.shape[2]), x.dtype), x[:, 0:t0 + TC, :]],
        axis=1)


@partial(jax.pmap, in_axes=(0, 0, 0, 0, None, None, None, None, None, None),
         out_axes=0)
def _shard_fn(xm, xr, qmask, kmask, w_mic, b_mic, w_ref, b_ref, w_conv, b_conv):
    # xm: (C, TQ, F) bf16, frames [t0-4, t0+128)
    # xr: (C, TK, F) bf16, frames [t0-35, t0+128)
    xrf = xr.astype(F32)
    Q = jnp.einsum('ctf,hc->htf', xm.astype(F32), w_mic,
                   preferred_element_type=F32) + b_mic[:, None, None]
    K = jnp.einsum('ctf,hc->htf', xrf, w_ref,
                   preferred_element_type=F32) + b_ref[:, None, None]
    Q = Q * qmask[None, :, None]   # zero frames before global t=0 (chunk 0)
    K = K * kmask[None, :, None]
    # V[h, t', d] = <Q[h, t'], K[h, t' + d]> / sqrt(F);  t' in [0, TQ)
    # (static slices instead of gather — the neuron compiler chokes on
    # indirect loads)
    V = jnp.stack(
        [jnp.sum(Q * jax.lax.slice_in_dim(K, d, d + TQ, axis=1), axis=-1)
         for d in range(DMAX)], axis=-1) / jnp.sqrt(F32(F))
    # conv (5,3) over (t', d), H->1; valid in t' (132->128), pad d by 1
    Vp = jnp.pad(V, ((0, 0), (0, 0), (1, 1)))[None]             # (1, H, TQ, 34)
    Vc = jax.lax.conv_general_dilated(
        Vp, w_conv, window_strides=(1, 1), padding='VALID',
        dimension_numbers=('NCHW', 'OIHW', 'NCHW'))[0, 0] + b_conv[0]
    A = jax.nn.softmax(Vc, axis=-1)                             # (TC, DMAX)
    # aligned[c, t, f] = sum_d A[t, d] * xr[c, t + 4 + d, f]
    out = jnp.zeros((C, TC, F), F32)
    for d in range(DMAX):
        out = out + A[None, :, d, None] * jax.lax.slice_in_dim(
            xrf, 4 + d, 4 + d + TC, axis=1)
    return out                                                  # (C, TC, F)


def _prep_shards(x_mic, x_ref):
    xm_s, xr_s, qm_s, km_s = [], [], [], []
    for b in range(B):
        for tc in range(NCHUNK):
            t0 = tc * TC
            xm_s.append(_shard_time(x_mic[b], t0, QHALO))
            xr_s.append(_shard_time(x_ref[b], t0, KHALO))
            qm = np.ones(TQ, np.float32)
            km = np.ones(TK, np.float32)
            if t0 - QHALO < 0:
                qm[:QHALO - t0] = 0.0
            if t0 - KHALO < 0:
                km[:KHALO - t0] = 0.0
            qm_s.append(qm)
            km_s.append(km)
    return (np.stack(xm_s), np.stack(xr_s), np.stack(qm_s), np.stack(km_s))


def kernel(x_mic, x_ref, w_mic, b_mic, w_ref, b_ref, w_conv, b_conv):
    x_mic = np.asarray(x_mic, np.float32)
    x_ref = np.asarray(x_ref, np.float32)
    xm, xr, qm, km = _prep_shards(x_mic, x_ref)
    out = _shard_fn(
        jnp.asarray(xm, jnp.bfloat16), jnp.asarray(xr, jnp.bfloat16),
        jnp.asarray(qm), jnp.asarray(km),
        jnp.asarray(w_mic, np.float32), jnp.asarray(b_mic, np.float32),
        jnp.asarray(w_ref, np.float32), jnp.asarray(b_ref, np.float32),
        jnp.asarray(w_conv, np.float32), jnp.asarray(b_conv, np.float32))
    out = np.asarray(out)             # (8, C, TC, F)
    full = np.empty((B, C, T, F), np.float32)
    for b in range(B):
        for tc in range(NCHUNK):
            full[b, :, tc * TC:(tc + 1) * TC, :] = out[b * NCHUNK + tc]
    return full


# revision 6
# speedup vs baseline: 1.4229x; 1.0246x over previous
"""Distributed AlignBlock kernel for 8 NeuronCores.

Sharding: data-parallel over B(2) x T-chunks(4 x 128) = 8 shards, one per
core. Each shard carries a causal halo (4 frames for the conv on the Q/V
side, 35 = 31 + 4 frames on the K / x_ref side). Weights are replicated.
All compute runs on the NeuronCores; the host only slices/pads inputs and
concatenates the 8 output shards. Input shards ship as bf16 (halves the
transfer), accumulation on device is f32.

Hardcoded problem shape: B=2, C=64, H=64, T=512, F=64, DMAX=32.
"""

import numpy as np
import jax
import jax.numpy as jnp
from functools import partial

B, C, H, T, F = 2, 64, 64, 512, 64
DMAX = 32
NCHUNK = 4          # T-chunks per batch element
TC = T // NCHUNK    # 128 frames per chunk
QHALO = 4           # conv reaches back 4 frames in t
KHALO = DMAX - 1 + QHALO  # 35: score window + conv halo
TQ = TC + QHALO     # 132 Q frames per shard
TK = TC + KHALO     # 163 K / x_ref frames per shard

F32 = jnp.float32


def _shard_time(x, t0, halo):
    """x: (C, T, F) -> (C, TC+halo, F) covering global frames [t0-halo, t0+TC),
    zero-padded where the range dips below 0."""
    lo = t0 - halo
    if lo >= 0:
        return x[:, lo:t0 + TC, :]
    pad = -lo
    return np.concatenate(
        [np.zeros((x.shape[0], pad, x.shape[2]), x.dtype), x[:, 0:t0 + TC, :]],
        axis=1)


@partial(jax.pmap, in_axes=(0, 0, 0, 0, None, None, None, None, None, None),
         out_axes=0)
def _shard_fn(xm, xr, qmask, kmask, w_mic, b_mic, w_ref, b_ref, w_conv, b_conv):
    # xm: (C, TQ, F) bf16, frames [t0-4, t0+128)
    # xr: (C, TK, F) bf16, frames [t0-35, t0+128)
    xrf = xr.astype(F32)
    Q = jnp.einsum('ctf,hc->htf', xm.astype(F32), w_mic,
                   preferred_element_type=F32) + b_mic[:, None, None]
    K = jnp.einsum('ctf,hc->htf', xrf, w_ref,
                   preferred_element_type=F32) + b_ref[:, None, None]
    Q = Q * qmask[None, :, None]   # zero frames before global t=0 (chunk 0)
    K = K * kmask[None, :, None]
    # V[h, t', d] = <Q[h, t'], K[h, t' + d]> / sqrt(F);  t' in [0, TQ)
    # One batched matmul for the full score matrix, then a gather-free band
    # extraction: reinterpreting the (TQ, TK) rows with row-length TK+1 puts
    # S[h, t, t+d] at position [t, d].
    S = jnp.einsum('htf,hsf->hts', Q, K, preferred_element_type=F32)
    Sflat = S.reshape(H, TQ * TK)
    Sflat = jnp.pad(Sflat, ((0, 0), (0, TQ)))
    V = Sflat.reshape(H, TQ, TK + 1)[:, :, :DMAX] / jnp.sqrt(F32(F))
    # conv (5,3) over (t', d), H->1; valid in t' (132->128), pad d by 1
    Vp = jnp.pad(V, ((0, 0), (0, 0), (1, 1)))[None]             # (1, H, TQ, 34)
    Vc = jax.lax.conv_general_dilated(
        Vp, w_conv, window_strides=(1, 1), padding='VALID',
        dimension_numbers=('NCHW', 'OIHW', 'NCHW'))[0, 0] + b_conv[0]
    A = jax.nn.softmax(Vc, axis=-1)                             # (TC, DMAX)
    # aligned[c, t, f] = sum_d A[t, d] * xr[c, t + 4 + d, f]
    # Build the banded mixing matrix M[t, s] = A[t, s - t - 4] with a
    # gather-free skew (pad + reshape with row length TK+TC-1), then one
    # batched matmul against x_ref.
    Apad = jnp.pad(A, ((0, 0), (4, TK - DMAX - 4)))             # (TC, TK)
    Z = jnp.pad(Apad, ((0, 0), (0, TC)))                        # (TC, TK+TC)
    M = Z.reshape(-1)[:TC * (TK + TC - 1)].reshape(
        TC, TK + TC - 1)[:, :TK]                                # (TC, TK)
    return jnp.einsum('ts,csf->ctf', M, xrf,
                      preferred_element_type=F32)               # (C, TC, F)


def _prep_shards(x_mic, x_ref):
    xm_s, xr_s, qm_s, km_s = [], [], [], []
    for b in range(B):
        for tc in range(NCHUNK):
            t0 = tc * TC
            xm_s.append(_shard_time(x_mic[b], t0, QHALO))
            xr_s.append(_shard_time(x_ref[b], t0, KHALO))
            qm = np.ones(TQ, np.float32)
            km = np.ones(TK, np.float32)
            if t0 - QHALO < 0:
                qm[:QHALO - t0] = 0.0
            if t0 - KHALO < 0:
                km[:KHALO - t0] = 0.0
            qm_s.append(qm)
            km_s.append(km)
    return (np.stack(xm_s), np.stack(xr_s), np.stack(qm_s), np.stack(km_s))


def kernel(x_mic, x_ref, w_mic, b_mic, w_ref, b_ref, w_conv, b_conv):
    x_mic = np.asarray(x_mic, np.float32)
    x_ref = np.asarray(x_ref, np.float32)
    xm, xr, qm, km = _prep_shards(x_mic, x_ref)
    out = _shard_fn(
        jnp.asarray(xm, jnp.bfloat16), jnp.asarray(xr, jnp.bfloat16),
        jnp.asarray(qm), jnp.asarray(km),
        jnp.asarray(w_mic, np.float32), jnp.asarray(b_mic, np.float32),
        jnp.asarray(w_ref, np.float32), jnp.asarray(b_ref, np.float32),
        jnp.asarray(w_conv, np.float32), jnp.asarray(b_conv, np.float32))
    out = np.asarray(out)             # (8, C, TC, F)
    full = np.empty((B, C, T, F), np.float32)
    for b in range(B):
        for tc in range(NCHUNK):
            full[b, :, tc * TC:(tc + 1) * TC, :] = out[b * NCHUNK + tc]
    return full
